# revision 13
# baseline (speedup 1.0000x reference)
"""CrossAttentionFusion kernel for 8x Trainium2 NeuronCores.

Sharding: data-parallel over batch B=8 -> one batch element per core.
No collectives needed; weights replicated to all cores.

Algebraic restructure vs the straightforward formulation (host-side,
weights-only precompute, fp64):
  logits = Q K^T = (Zq Wq^T + bq)(Zk Wk^T)^T = Zq G Zk^T + 1 (bq Wk) Zk^T
with G = Wq^T Wk.  So the K projection disappears entirely (keys are the
raw Zk^T already resident in SBUF), and the per-key bias term folds into
the Y = Zq G + (bq Wk) projection as a per-partition ACT bias in the
Y^T eviction.  Saves 2 of the 7 projection matmuls and their evictions.

Precision: matmuls bf16 with fp32 PSUM accumulation, EXCEPT the big
S^T = Zk^T . Y^T score matmuls which run fp8-e4m3 with
perf_mode=DoubleRow (256-row contraction per pass, ~1.8x issue rate):
 - Zk^T quantized host-side (x16, one rounding off the raw input),
 - Y^T evicted to e4m3 (x64 via ACT scale, bias = 64*(bq Wk)),
 - exp eviction folds 1/(sqrt(D)*16*64) into the ACT scale.
Simulated end-to-end rel err (absmax/absmax metric): 1.26e-2 vs the
2e-2 gate (bf16 baseline was 2.96e-3).

Other bias folds (host-side): V biases pass through softmax unchanged
-> folded into the final bias: bf_eff = bf + Wf @ (bvl + bvg), fp32.

Per-core dataflow (S=2048 seq, D=768 model dim):
  - 28 warmup matmuls bridge the initial DMA window (HAM clock-gate).
  - Per direction (g2l, l2g):
      V[k,d] projected over the full sequence (ACT copy evictions).
      Per q-block of 512:
        Y^T[e,q] projected on the fly -> e4m3 eviction with bias.
        S^T[k,q] = sum_e Zk8^T(pair) . Y8^T in 3 DoubleRow matmuls
        -> exp on ScalarE (scale folded) -> E^T bf16.  Softmax
        denominator accumulated on the VectorE (fp32 ping-pong adds).
        U^T[d,q] = sum_k V(chunk) . E^T accumulated in PSUM in two
        3-bank passes; banks evicted UNNORMALIZED (ACT copy, bf16).
        Normalization: one fp32 partition-sum matmul -> DVE reciprocal
        -> K=1 fp32 broadcast matmul -> DVE multiply.
  - dir0 result (bf16, Z^T layout) stays SBUF-resident (the freed K^T
    space); dir1 adds its contribution and runs the final projection
    software-pipelined one q-block behind, writing fp32 rows.
"""

import numpy as np
import ml_dtypes

import concourse.bass as bass
import concourse.mybir as mybir
import concourse.tile as tile
from concourse import bacc
from concourse.bass_utils import run_bass_kernel_spmd

S = 2048
D = 768
P = 128
NDC = D // P      # 6 chunks of the model dim
NSC = S // P      # 16 chunks of the sequence
QB = 512          # q-block width
NQB = S // QB     # 4 q-blocks
NH = 2            # halves of D for N=384 matmuls
HWID = D // NH    # 384
NCORES = 8
INV_SQRT_D = float(1.0 / np.sqrt(D))

S_Z = 16.0        # host-side e4m3 scale on Z^T
S_Y = 64.0        # ACT-side e4m3 scale on Y^T
EXP_SCALE = float(INV_SQRT_D / (S_Z * S_Y))

F32 = mybir.dt.float32
BF16 = mybir.dt.bfloat16
FP8 = mybir.dt.float8e4
DR = mybir.MatmulPerfMode.DoubleRow

# (kv_src, q_src) per direction; sources index (zg, zl)
DIRS = [(1, 0),   # graph queries attend lstm keys/values
        (0, 1)]   # lstm queries attend graph keys/values

AF = mybir.ActivationFunctionType


def build_kernel_body(nc, tc, zt_dram, z8_dram, g_dram, wv_dram, wf_dram,
                      ub_dram, br_dram, out_ap):
    import contextlib
    with contextlib.ExitStack() as stk:
        persist = stk.enter_context(tc.tile_pool(name="persist", bufs=1))
        psum = stk.enter_context(tc.tile_pool(name="psum", bufs=1, space="PSUM"))
        work = stk.enter_context(tc.tile_pool(name="work", bufs=1))

        # ---- constants ----
        ones_col = persist.tile([P, 1], BF16, name="ones_col", tag="ones_col")
        nc.vector.memset(ones_col[:], 1.0)
        ones_row_f = persist.tile([1, P], F32, name="ones_row_f", tag="ones_row_f")
        nc.vector.memset(ones_row_f[:], 1.0)
        ones_row_b = persist.tile([1, P], BF16, name="ones_row_b", tag="ones_row_b")
        nc.vector.memset(ones_row_b[:], 1.0)

        # ---- PE warmup asap (HAM clock-gate), before any DMA deps ----
        wu = work.tile([P, QB], BF16, name="wu", tag="wu", bufs=1)
        nc.vector.memset(wu[:], 0.0)
        for i in range(28):
            wps = psum.tile([P, QB], F32, name=f"wps{i}", tag="S", bufs=7)
            nc.tensor.matmul(wps[:], lhsT=wu[:, 0:P], rhs=wu[:],
                             start=True, stop=True)

        # ---- small parameter tensors (ScalarE HWDGE queue: keeps the
        # GpSimd SWDGE queue head free for the big Z transfers) ----
        ub_sb = []
        for di in range(2):
            t = persist.tile([P, NDC], F32, name=f"ub_{di}", tag=f"ub_{di}")
            nc.scalar.dma_start(out=t[:], in_=ub_dram[di][:, :])
            ub_sb.append(t)
        br_sb = persist.tile([1, D], F32, name="br_wf", tag="br_wf")
        nc.scalar.dma_start(out=br_sb[:], in_=br_dram[:, :])
        # fp32 broadcast of the (folded) final bias across partitions
        bias_bc = persist.tile([P, D], F32, name="bias_bc", tag="bias_bc")
        for h in range(NH):
            bps = psum.tile([P, HWID], F32, name=f"bps{h}", tag="S", bufs=7)
            nc.tensor.matmul(bps[:], lhsT=ones_row_f[:],
                             rhs=br_sb[0:1, h * HWID:(h + 1) * HWID],
                             start=True, stop=True)
            nc.vector.tensor_copy(bias_bc[:, h * HWID:(h + 1) * HWID], bps[:])

        # final projection weight, persistent (used in dir1 inner loop);
        # DMA emitted after dir0's weights (queue order = first use)
        wf_sb = [persist.tile([P, D], BF16, name=f"wf_{dc}", tag=f"wf_{dc}")
                 for dc in range(NDC)]

        # ---- Z^T (bf16, d on partitions) + e4m3 copies (x16), both
        # host-pretransposed/precast.  Load z_lstm first (dir0 projects
        # V from it), in q-block column chunks so compute starts early.
        zt = [[persist.tile([P, S], BF16, name=f"zt{si}_{dc}", tag=f"zt{si}_{dc}")
               for dc in range(NDC)] for si in range(2)]
        z8 = [persist.tile([P, NDC * S], FP8, name=f"z8_{si}", tag=f"z8_{si}")
              for si in range(2)]
        # first V/K source block via the ScalarE HWDGE queue (lands first);
        # the rest as few big SWDGE transfers (descriptor issue on GpSimd
        # is ~0.6us each, so fewer is better).
        for sb in range(2):
            for dc in range(NDC):
                nc.scalar.dma_start(
                    out=zt[1][dc][:, sb * QB:(sb + 1) * QB],
                    in_=zt_dram[1][dc * P:(dc + 1) * P, sb * QB:(sb + 1) * QB])
        for dc in range(NDC):
            nc.gpsimd.dma_start(out=zt[1][dc][:, 2 * QB:S],
                                in_=zt_dram[1][dc * P:(dc + 1) * P, 2 * QB:S])
        for dc in range(NDC):
            nc.gpsimd.dma_start(out=zt[0][dc][:],
                                in_=zt_dram[0][dc * P:(dc + 1) * P, :])
        # e4m3 keys: dir0 needs z8[1] (keys = z_lstm), dir1 needs z8[0]
        for si in (1, 0):
            for dc in range(NDC):
                nc.gpsimd.dma_start(
                    out=z8[si][:, dc * S:(dc + 1) * S],
                    in_=z8_dram[si][dc * P:(dc + 1) * P, :])

        # DRAM scratch holding dir0's normalized output in Z^T layout (bf16)
        dram = stk.enter_context(tc.tile_pool(name="dram", bufs=1, space="DRAM"))
        zfg_dram = dram.tile([D, S], BF16, name="zfg_scratch", tag="zfg")

        # dir1's G and Wv prefetched into long-lived tiles during dir0
        g1_sb = [work.tile([P, D], BF16, name=f"g1_{dc}", tag=f"g1_{dc}",
                           bufs=1) for dc in range(NDC)]
        wv1_sb = [work.tile([P, D], BF16, name=f"wv1_{dc}", tag=f"wv1_{dc}",
                            bufs=1) for dc in range(NDC)]

        # ---- the two attention directions ----
        for di, (kv_src, q_src) in enumerate(DIRS):
            with tc.tile_pool(name=f"dir{di}", bufs=1) as dp:
                if di == 0:
                    g_sb = [dp.tile([P, D], BF16, name=f"g0_{dc}",
                                    tag=f"g_{dc}") for dc in range(NDC)]
                    wv_sb = [dp.tile([P, D], BF16, name=f"wv0_{dc}",
                                     tag=f"wv_{dc}") for dc in range(NDC)]
                    for dc in range(NDC):
                        nc.sync.dma_start(out=wv_sb[dc][:],
                                          in_=wv_dram[0][dc * P:(dc + 1) * P, :])
                    for dc in range(NDC):
                        nc.sync.dma_start(out=g_sb[dc][:],
                                          in_=g_dram[0][dc * P:(dc + 1) * P, :])
                    # prefetch dir1's weights + Wf while dir0 computes
                    for dc in range(NDC):
                        nc.sync.dma_start(out=wv1_sb[dc][:],
                                          in_=wv_dram[1][dc * P:(dc + 1) * P, :])
                    for dc in range(NDC):
                        nc.sync.dma_start(out=g1_sb[dc][:],
                                          in_=g_dram[1][dc * P:(dc + 1) * P, :])
                    for dc in range(NDC):
                        nc.sync.dma_start(out=wf_sb[dc][:],
                                          in_=wf_dram[dc * P:(dc + 1) * P, :])
                else:
                    g_sb, wv_sb = g1_sb, wv1_sb

                # e4m3 keys for this direction, as a [P, 2, P]-sliceable view
                z8k = z8[kv_src][:].rearrange("p (c s) -> p c s", c=NDC)

                # ---- V[s, e] natural layout ----
                v_sb = [dp.tile([P, D], BF16, name=f"v{di}_{sc}", tag=f"v_{sc}")
                        for sc in range(NSC)]
                for sc in range(NSC):
                    for h in range(NH):
                        ps = psum.tile([P, HWID], F32, name=f"ps_v{sc}_{h}",
                                       tag="S", bufs=7)
                        for dc in range(NDC):
                            nc.tensor.matmul(
                                ps[:],
                                lhsT=zt[kv_src][dc][:, sc * P:(sc + 1) * P],
                                rhs=wv_sb[dc][:, h * HWID:(h + 1) * HWID],
                                start=(dc == 0), stop=(dc == NDC - 1))
                        nc.scalar.activation(
                            v_sb[sc][:, h * HWID:(h + 1) * HWID], ps[:], AF.Copy)

                # ---- attention, one q-block at a time ----
                # final projection (dir1) runs one q-block behind; pend holds
                # the normalized+summed z_fused^T tiles of the previous block.
                pend = None

                def final_proj(zfqb, qb):
                    for i in range(QB // P):
                        ostage = work.tile([P, D], F32, name=f"os{qb}_{i}",
                                           tag="ostage", bufs=2)
                        for h in range(NH):
                            fp = psum.tile([P, HWID], F32, name=f"fp{qb}_{i}_{h}",
                                           tag="S", bufs=7)
                            for dc in range(NDC):
                                nc.tensor.matmul(
                                    fp[:], lhsT=zfqb[dc][:, i * P:(i + 1) * P],
                                    rhs=wf_sb[dc][:, h * HWID:(h + 1) * HWID],
                                    start=(dc == 0), stop=(dc == NDC - 1))
                            nc.vector.tensor_add(
                                ostage[:, h * HWID:(h + 1) * HWID], fp[:],
                                bias_bc[:, h * HWID:(h + 1) * HWID])
                        row0 = qb * QB + i * P
                        nc.sync.dma_start(out=out_ap[row0:row0 + P, :],
                                          in_=ostage[:])

                for qb in range(NQB):
                    if di == 1:
                        zfg_in = []
                        for dc in range(NDC):
                            zin = work.tile([P, QB], BF16, name=f"zfi{qb}_{dc}",
                                            tag="zfg_in", bufs=5)
                            nc.sync.dma_start(
                                out=zin[:],
                                in_=zfg_dram[dc * P:(dc + 1) * P,
                                             qb * QB:(qb + 1) * QB])
                            zfg_in.append(zin)

                    # Y^T for this q-block: Y = Zq G + (bq Wk), evicted e4m3
                    y8 = work.tile([P, NDC * QB], FP8, name=f"y8{qb}",
                                   tag="y8", bufs=2)
                    for ec in range(NDC):
                        ps = psum.tile([P, QB], F32, name=f"ps_y{qb}_{ec}",
                                       tag="S", bufs=7)
                        for dc in range(NDC):
                            nc.tensor.matmul(
                                ps[:],
                                lhsT=g_sb[dc][:, ec * P:(ec + 1) * P],
                                rhs=zt[q_src][dc][:, qb * QB:(qb + 1) * QB],
                                start=(dc == 0), stop=(dc == NDC - 1))
                        nc.scalar.activation(
                            y8[:, ec * QB:(ec + 1) * QB], ps[:], AF.Identity,
                            bias=ub_sb[di][:, ec:ec + 1], scale=S_Y)
                    y8v = y8[:].rearrange("p (c q) -> p c q", c=NDC)

                    # previous q-block's final projection (PE-dense filler
                    # while this block's S-phase evictions run on ACT/DVE)
                    if pend is not None:
                        final_proj(*pend)
                        pend = None

                    # S^T chunks (fp8 DoubleRow), batched contiguously: the
                    # PE pays ~190ns per bf16<->DoubleRow mode switch, so all
                    # 48 fp8 matmuls run back-to-back.  exp + the fp32
                    # denominator chain (VectorE) trail the sp banks.
                    e_tiles = []
                    racc = None
                    for kc in range(NSC):
                        sp = psum.tile([P, QB], F32, name=f"s{qb}_{kc}",
                                       tag="S", bufs=7)
                        for c in range(NDC // 2):
                            nc.tensor.matmul(
                                sp[:],
                                lhsT=z8k[:, 2 * c:2 * c + 2,
                                         kc * P:(kc + 1) * P],
                                rhs=y8v[:, 2 * c:2 * c + 2, :],
                                start=(c == 0), stop=(c == NDC // 2 - 1),
                                perf_mode=DR)
                        et = work.tile([P, QB], BF16, name=f"et{qb}_{kc}",
                                       tag="et", bufs=16)
                        nc.scalar.activation(et[:], sp[:], AF.Exp,
                                             scale=EXP_SCALE)
                        e_tiles.append(et)
                        ra = work.tile([P, QB], F32, name=f"ra{qb}_{kc}",
                                       tag="racc", bufs=2)
                        if racc is None:
                            nc.vector.tensor_copy(ra[:], et[:])
                        else:
                            nc.vector.tensor_add(ra[:], racc[:], et[:])
                        racc = ra
                    # final racc in bf16 so the partition-sum matmul runs at
                    # the 1 cyc/col bf16 rate (fp32 matmul is 4 cyc/col)
                    racc_b = work.tile([P, QB], BF16, name=f"rab{qb}",
                                       tag="racc_b", bufs=1)
                    nc.vector.tensor_copy(racc_b[:], racc[:])

                    # U pass 1 (bf16, contiguous)
                    u_ps = [psum.tile([P, QB], F32, name=f"u{qb}_{dc}",
                                      tag="S", bufs=7) for dc in range(3)]
                    for kc in range(NSC):
                        for dc in range(3):
                            nc.tensor.matmul(
                                u_ps[dc][:],
                                lhsT=v_sb[kc][:, dc * P:(dc + 1) * P],
                                rhs=e_tiles[kc][:],
                                start=(kc == 0), stop=(kc == NSC - 1))

                    # unnormalized evictions of pass 1 (frees pu banks fast)
                    usb = [None] * NDC
                    for dc in range(NDC):
                        usb[dc] = work.tile([P, QB], BF16, name=f"usb{qb}_{dc}",
                                            tag="usb", bufs=6)
                    for dc in range(3):
                        nc.scalar.activation(usb[dc][:], u_ps[dc][:], AF.Copy)

                    # U pass 2; the single fp32 partition-sum matmul for the
                    # denominator is slotted after the first U2 chunk so its
                    # DVE-chain dependency and the reciprocal latency hide
                    # under the remaining matmuls.
                    r_ps = psum.tile([1, QB], F32, name=f"r{qb}", tag="r", bufs=1)
                    rsb = work.tile([1, QB], BF16, name=f"rsb{qb}", tag="rsb",
                                    bufs=1)
                    u_ps2 = [psum.tile([P, QB], F32, name=f"u2{qb}_{dc}",
                                       tag="S", bufs=7) for dc in range(3)]
                    for kc in range(NSC):
                        for i, dc in enumerate(range(3, NDC)):
                            nc.tensor.matmul(
                                u_ps2[i][:],
                                lhsT=v_sb[kc][:, dc * P:(dc + 1) * P],
                                rhs=e_tiles[kc][:],
                                start=(kc == 0), stop=(kc == NSC - 1))
                        if kc == 0:
                            nc.tensor.matmul(r_ps[0:1, :], lhsT=ones_col[:],
                                             rhs=racc_b[:], start=True, stop=True)
                            with nc.allow_low_precision(
                                    reason="1/r in bf16: 0.1% rms, checked in sim"):
                                nc.vector.reciprocal(rsb[:], r_ps[0:1, :])
                    for i, dc in enumerate(range(3, NDC)):
                        nc.scalar.activation(usb[dc][:], u_ps2[i][:], AF.Copy)

                    # broadcast 1/r across partitions (reciprocal done by now);
                    # bf16 all the way so the matmul runs at bf16 rate and the
                    # DVE normalize multiplies get the 2x 16-bit path.
                    rb_ps = psum.tile([P, QB], F32, name=f"rb{qb}", tag="r", bufs=1)
                    nc.tensor.matmul(rb_ps[:], lhsT=ones_row_b[:], rhs=rsb[:],
                                     start=True, stop=True)
                    rb_sb = work.tile([P, QB], BF16, name=f"rbs{qb}", tag="rb_sb",
                                      bufs=1)
                    nc.vector.tensor_copy(rb_sb[:], rb_ps[:])

                    # normalize (+ combine with dir0 for dir1)
                    if di == 0:
                        for dc in range(NDC):
                            zst = work.tile([P, QB], BF16, name=f"zst{qb}_{dc}",
                                            tag="zst", bufs=2)
                            nc.vector.tensor_mul(zst[:], usb[dc][:], rb_sb[:])
                            nc.sync.dma_start(
                                out=zfg_dram[dc * P:(dc + 1) * P,
                                             qb * QB:(qb + 1) * QB],
                                in_=zst[:])
                    else:
                        zfqb = [None] * NDC
                        for dc in range(NDC):
                            zm = work.tile([P, QB], BF16, name=f"zm{qb}_{dc}",
                                           tag="zfqb_m", bufs=2)
                            nc.vector.tensor_mul(zm[:], usb[dc][:], rb_sb[:])
                            zs = work.tile([P, QB], BF16, name=f"zf{qb}_{dc}",
                                           tag="zfqb", bufs=8)
                            nc.vector.tensor_add(zs[:], zm[:], zfg_in[dc][:])
                            zfqb[dc] = zs
                        pend = (zfqb, qb)

                if pend is not None:
                    final_proj(*pend)
                    pend = None


_CACHED = {}


def _build_nc():
    if "nc" in _CACHED:
        return _CACHED["nc"]
    nc = bacc.Bacc("TRN2", target_bir_lowering=False, debug=False)
    ztg = nc.dram_tensor("zt_graph", [D, S], BF16, kind="ExternalInput")
    ztl = nc.dram_tensor("zt_lstm", [D, S], BF16, kind="ExternalInput")
    z8g = nc.dram_tensor("z8_graph", [D, S], FP8, kind="ExternalInput")
    z8l = nc.dram_tensor("z8_lstm", [D, S], FP8, kind="ExternalInput")
    g = [nc.dram_tensor(f"g_{di}", [D, D], BF16, kind="ExternalInput")
         for di in range(2)]
    wv = [nc.dram_tensor(f"wv_{di}", [D, D], BF16, kind="ExternalInput")
          for di in range(2)]
    wf = nc.dram_tensor("wf", [D, D], BF16, kind="ExternalInput")
    ub = [nc.dram_tensor(f"ub_{di}", [P, NDC], F32, kind="ExternalInput")
          for di in range(2)]
    br = nc.dram_tensor("br_wf", [1, D], F32, kind="ExternalInput")
    out = nc.dram_tensor("out", [S, D], F32, kind="ExternalOutput")

    with tile.TileContext(nc) as tc:
        build_kernel_body(
            nc, tc, (ztg.ap(), ztl.ap()), (z8g.ap(), z8l.ap()),
            [x.ap() for x in g], [x.ap() for x in wv], wf.ap(),
            [x.ap() for x in ub], br.ap(), out.ap(),
        )
    nc.compile()
    _CACHED["nc"] = nc
    return nc


def make_in_maps(inputs):
    """Host-side sharding: one batch element per core; weights replicated.
    Weight-only precompute (fp64): G = Wq^T Wk, u = bq Wk, and the V/final
    bias fold bf_eff = bf + Wf (bvl + bvg).  Z pre-transposed and cast to
    bf16 + e4m3(x16)."""
    bf16 = ml_dtypes.bfloat16
    e4 = ml_dtypes.float8_e4m3
    zg = np.asarray(inputs["Z_graph"], dtype=np.float32)
    zl = np.asarray(inputs["Z_lstm"], dtype=np.float32)
    W64 = {n: np.asarray(inputs[n], dtype=np.float64)
           for n in ("Wqg", "Wkl", "Wvl", "Wql", "Wkg", "Wvg", "Wf")}
    shared = {}
    # direction 0: graph queries, lstm keys/values.  direction 1: reverse.
    for di, (wq, bq, wk, wv_) in enumerate(
            [("Wqg", "bqg", "Wkl", "Wvl"), ("Wql", "bql", "Wkg", "Wvg")]):
        G = (W64[wq].T @ W64[wk]).astype(np.float32)
        u = (np.asarray(inputs[bq], np.float64) @ W64[wk]).astype(np.float32)
        shared[f"g_{di}"] = np.ascontiguousarray(G).astype(bf16)
        shared[f"ub_{di}"] = np.ascontiguousarray(
            (S_Y * u).reshape(NDC, P).T.astype(np.float32))
        shared[f"wv_{di}"] = np.ascontiguousarray(W64[wv_].T).astype(bf16)
    shared["wf"] = np.ascontiguousarray(W64["Wf"].T).astype(bf16)
    # K biases are softmax-invariant (constant per query row) -> dropped.
    # V biases pass through attention unchanged (softmax rows sum to 1),
    # so they fold into the final bias: bf_eff = bf + Wf @ (bvl + bvg).
    bf_eff = (np.asarray(inputs["bf"], dtype=np.float64)
              + W64["Wf"] @ (np.asarray(inputs["bvl"], np.float64)
                             + np.asarray(inputs["bvg"], np.float64)))
    shared["br_wf"] = np.ascontiguousarray(
        bf_eff.astype(np.float32).reshape(1, D))
    in_maps = []
    for c in range(NCORES):
        m = dict(shared)
        ztg = np.ascontiguousarray(zg[c].T)
        ztl = np.ascontiguousarray(zl[c].T)
        m["zt_graph"] = ztg.astype(bf16)
        m["zt_lstm"] = ztl.astype(bf16)
        m["z8_graph"] = (ztg * np.float32(S_Z)).astype(e4)
        m["z8_lstm"] = (ztl * np.float32(S_Z)).astype(e4)
        in_maps.append(m)
    return in_maps


def run(inputs, trace=False, **kwargs):
    nc = _build_nc()
    in_maps = make_in_maps(inputs)
    res = run_bass_kernel_spmd(nc, in_maps, list(range(NCORES)),
                               trace=trace, **kwargs)
    out = np.stack([res.results[c]["out"] for c in range(NCORES)], axis=0)
    return out.astype(np.float32), res


def kernel(**inputs):
    out, _ = run(inputs, trace=False)
    return out
